# revision 1
# baseline (speedup 1.0000x reference)
"""LinearWithLoRA on 8 TRN2 NeuronCores.

y = x @ W.T + b + 2.0 * (x @ A.T) @ B.T
  x: [4, 2048, 2048] f32, W: [2048, 2048], b: [2048], A: [16, 2048], B: [2048, 16]

Strategy:
- LoRA merge on host: W' = W + 2.0 * B @ A (134 MFLOP on CPU, the standard
  LoRA deployment fold), so the device computes just x @ W'.T + b.
- Data-parallel over tokens (8192 tokens -> 1024 per core). Each core reads
  its x shard + the full replicated W'/b and writes its out shard; no
  collectives. This minimizes DMA: 8 (x) + 16 (W') + 8 (out) MiB per core.
- Host pre-transposes x and W' so both matmul operands are K(=d_in)-major in
  DRAM: no on-device transposes, every DMA is >=2KB-contiguous per partition.
- Matmuls run in float32r (TF32-like, full PE rate for moving dim >= 256,
  ~1e-4 rel err at K=2048); fp32 would be 4x slower on the PE.
- Bias is added in exact fp32 during PSUM->SBUF eviction on the vector
  engine, then stored straight to the out shard layout.
"""

import numpy as np

import concourse.bass as bass
import concourse.mybir as mybir
import concourse.tile as tile
from concourse import bacc
from concourse.bass import ds, ts
from concourse.bass_utils import run_bass_kernel_spmd

B, S, D_IN, D_OUT, R = 4, 2048, 2048, 2048, 16
SCALING = 32.0 / 16.0
N_CORES = 8
TOK = B * S  # 8192
TOK_SHARD = TOK // N_CORES  # 1024
P = 128
KO = D_IN // P  # 16 contraction tiles
N_CHUNK = 512  # psum bank limit for f32 moving operand
N_CHUNKS = D_OUT // N_CHUNK  # 4
M_TILES = TOK_SHARD // P  # 8

_nc_cache = {}


def _build(reps=1, timing=False):
    f32 = mybir.dt.float32
    f32r = mybir.dt.float32r

    nc = bacc.Bacc(None, target_bir_lowering=False)
    xT = nc.dram_tensor("xT", [D_IN, TOK_SHARD], f32r, kind="ExternalInput")
    wT = nc.dram_tensor("wT", [D_IN, D_OUT], f32r, kind="ExternalInput")
    bias = nc.dram_tensor("bias", [1, D_OUT], f32, kind="ExternalInput")
    if timing:
        nc.dram_tensor("tiny_out", [1, 1], f32, kind="ExternalOutput")
        out = nc.dram_tensor("oscratch", [TOK_SHARD, D_OUT], f32)  # internal
    else:
        out = nc.dram_tensor("out", [TOK_SHARD, D_OUT], f32, kind="ExternalOutput")

    xT3 = xT.rearrange("(ko p) t -> p ko t", p=P)
    wT3 = wT.rearrange("(ko p) n -> p ko n", p=P)

    with tile.TileContext(nc) as tc:
        with (
            tc.tile_pool(name="xpool", bufs=1) as xpool,
            tc.tile_pool(name="wpool", bufs=2) as wpool,
            tc.tile_pool(name="cpool", bufs=1) as cpool,
            tc.tile_pool(name="opool", bufs=4) as opool,
            tc.tile_pool(name="ppool", bufs=8, space="PSUM") as ppool,
        ):
            # x shard, fully resident: [128, 16, 1024] = 64 KB/partition.
            xt = xpool.tile([P, KO, TOK_SHARD], f32r)
            bias_t = cpool.tile([P, D_OUT], f32)

            def load_wt(n):
                wt = wpool.tile([P, KO, N_CHUNK], f32r, tag="w")
                for kg in range(4):  # 4 k-groups of 4 -> 1 MiB DMAs
                    nc.sync.dma_start(
                        out=wt[:, ds(kg * 4, 4), :],
                        in_=wT3[:, ds(kg * 4, 4), ts(n, N_CHUNK)],
                    )
                return wt

            if timing:
                # Timing build: slope over repeated main phases; prologue
                # order doesn't matter, load everything up front.
                for k in range(KO):
                    nc.sync.dma_start(out=xt[:, k, :], in_=xT3[:, k, :])
                nc.sync.dma_start(
                    out=bias_t[:], in_=bias[:].to_broadcast((P, D_OUT))
                )
                first_wt = None
            else:
                # Single-shot prologue: interleave the n=0 W chunk with the
                # x k-stream (HWDGE issues in FIFO order) so the k-major
                # matmul stream can start after ~1.5 MiB instead of
                # waiting behind the full 8 MiB x load.
                first_wt = wpool.tile([P, KO, N_CHUNK], f32r, tag="w")
                for kg in range(4):
                    nc.sync.dma_start(
                        out=first_wt[:, ds(kg * 4, 4), :],
                        in_=wT3[:, ds(kg * 4, 4), ts(0, N_CHUNK)],
                    )
                    for k in range(4 * kg, 4 * kg + 4):
                        nc.sync.dma_start(out=xt[:, k, :], in_=xT3[:, k, :])
                nc.sync.dma_start(
                    out=bias_t[:], in_=bias[:].to_broadcast((P, D_OUT))
                )

            # Main phase: stream W' by dout chunk; x stays resident. Matmuls
            # run k-major across all 8 m-groups (8 PSUM banks) so PE consumes
            # x/W chunks in stream-arrival order.
            def evict(ps, n, m):
                ot = opool.tile([P, N_CHUNK], f32, name="ot")
                nc.vector.tensor_add(ot[:], ps[:], bias_t[:, ts(n, N_CHUNK)])
                nc.sync.dma_start(out=out[ts(m, P), ts(n, N_CHUNK)], in_=ot[:])

            def main_phase(_iv=None, first_wt=None):
                for n in range(N_CHUNKS):
                    wt = first_wt if (n == 0 and first_wt is not None) else load_wt(n)
                    if n == 0 and first_wt is not None:
                        # k-major across all 8 m-groups (8 PSUM banks): PE
                        # consumes the interleaved x/W prologue streams in
                        # arrival order.
                        pss = [
                            ppool.tile([P, N_CHUNK], f32, tag="ps", name=f"ps{m}")
                            for m in range(M_TILES)
                        ]
                        for k in range(KO):
                            for m in range(M_TILES):
                                nc.tensor.matmul(
                                    pss[m][:],
                                    xt[:, k, ts(m, P)],
                                    wt[:, k, :],
                                    start=(k == 0),
                                    stop=(k == KO - 1),
                                )
                        for m in range(M_TILES):
                            evict(pss[m], n, m)
                    else:
                        # m-major: group completions stagger, so evictions
                        # and output stores overlap the matmul stream.
                        for m in range(M_TILES):
                            ps = ppool.tile([P, N_CHUNK], f32, tag="ps", name="ps")
                            for k in range(KO):
                                nc.tensor.matmul(
                                    ps[:],
                                    xt[:, k, ts(m, P)],
                                    wt[:, k, :],
                                    start=(k == 0),
                                    stop=(k == KO - 1),
                                )
                            evict(ps, n, m)

            if timing and reps > 1:
                tc.For_i_unrolled(0, reps, 1, main_phase, max_unroll=4)
            else:
                main_phase(first_wt=first_wt)

    nc.compile()
    return nc


def _make_in_maps(x, W, b, lora_A, lora_B):
    # LoRA merge: W' = W + scaling * B @ A  (exact fp32 host math)
    w_merged = W + SCALING * (lora_B @ lora_A)
    xT = np.ascontiguousarray(x.reshape(TOK, D_IN).T)  # [D_IN, TOK]
    wT = np.ascontiguousarray(w_merged.T)  # [D_IN, D_OUT]
    bias = np.ascontiguousarray(b[None, :])  # [1, D_OUT]
    return [
        {
            "xT": np.ascontiguousarray(xT[:, i * TOK_SHARD : (i + 1) * TOK_SHARD]),
            "wT": wT,
            "bias": bias,
        }
        for i in range(N_CORES)
    ]


def kernel(x, W, b, lora_A, lora_B):
    x = np.asarray(x, dtype=np.float32)
    W = np.asarray(W, dtype=np.float32)
    b = np.asarray(b, dtype=np.float32)
    lora_A = np.asarray(lora_A, dtype=np.float32)
    lora_B = np.asarray(lora_B, dtype=np.float32)

    if "main" not in _nc_cache:
        _nc_cache["main"] = _build()
    nc = _nc_cache["main"]

    in_maps = _make_in_maps(x, W, b, lora_A, lora_B)
    res = run_bass_kernel_spmd(nc, in_maps, list(range(N_CORES)))
    out = np.concatenate([res.results[i]["out"] for i in range(N_CORES)], axis=0)
    return out.reshape(B, S, D_OUT)



# revision 2
# speedup vs baseline: 1.0597x; 1.0597x over previous
"""LinearWithLoRA on 8 TRN2 NeuronCores — fp8 DoubleRow tensor-parallel-free
data-parallel kernel.

y = x @ W.T + b + 2.0 * (x @ A.T) @ B.T
  x: [4, 2048, 2048] f32, W: [2048, 2048], b: [2048], A: [16, 2048], B: [2048, 16]

Strategy:
- LoRA merge on host: W' = W + 2.0 * B @ A (standard LoRA deployment fold),
  so the device computes x @ W'.T + b.
- Data-parallel over tokens (8192 -> 1024/core); W' replicated. No
  collectives.
- fp8 e4m3 DoubleRow matmuls (2 k-planes per instruction at 0.5 cycles/row)
  double the PE throughput vs fp32r/bf16. Quantization: x8 = q(x*32),
  W8 = q(W'*1024), both power-of-2 scales so dequant is exact.
- Precision recovery: pure-fp8 error is 2.4e-2 (> 2e-2 gate). The x-side
  residual dx8 = q(x*32 - x8) is quantized at the SAME scale and simply
  accumulated into the same PSUM group (8 extra matmuls per tile), which
  cancels the x quantization error: measured rel err 1.71e-2.
- Operand swap vs the usual layout: W chunk is the stationary operand, x the
  moving one, so PSUM tiles come out as [out_ch(partition), tokens(free)].
  The bias then varies along partitions and the whole eviction is ONE
  scalar-engine op per tile: out = Identity(psum * 2^-15 + bias[p]).
  Vector engine stays idle; output is written transposed [D_OUT, TOK] and
  re-transposed on host.
- Host pre-layouts give fully contiguous DMA lines: x8/dx8 are single
  16 KiB/partition DMAs, W8 streams in 16 chunks of 2 KiB/partition.
"""

import numpy as np
import ml_dtypes

import concourse.bass as bass
import concourse.mybir as mybir
import concourse.tile as tile
from concourse import bacc
from concourse.bass import ds, ts
from concourse.bass_utils import run_bass_kernel_spmd

B, S, D_IN, D_OUT, R = 4, 2048, 2048, 2048, 16
SCALING = 32.0 / 16.0
N_CORES = 8
TOK = B * S  # 8192
TOK_SHARD = TOK // N_CORES  # 1024
P = 128
KO2 = D_IN // 256  # 8 DoubleRow contraction tiles (256 each)
T_CHUNK = 512  # psum bank: 512 f32 per partition
T_CHUNKS = TOK_SHARD // T_CHUNK  # 2
O_TILES = D_OUT // P  # 16 out-channel tiles

SX = 32.0  # x scale (2^5)
SW = 1024.0  # W scale (2^10)
EVICT_SCALE = 1.0 / (SX * SW)  # 2^-15, exact in fp32

E4M3 = ml_dtypes.float8_e4m3

_nc_cache = {}


def _build():
    f32 = mybir.dt.float32
    f8 = mybir.dt.float8e4
    DR = mybir.MatmulPerfMode.DoubleRow
    IDENT = mybir.ActivationFunctionType.Identity

    nc = bacc.Bacc(None, target_bir_lowering=False)
    x8d = nc.dram_tensor("x8", [P, KO2, 2, TOK_SHARD], f8, kind="ExternalInput")
    dx8d = nc.dram_tensor("dx8", [P, KO2, 2, TOK_SHARD], f8, kind="ExternalInput")
    w8d = nc.dram_tensor("w8", [O_TILES, P, KO2, 2, P], f8, kind="ExternalInput")
    biasd = nc.dram_tensor("bias", [P, O_TILES], f32, kind="ExternalInput")
    out = nc.dram_tensor("outT", [D_OUT, TOK_SHARD], f32, kind="ExternalOutput")

    with tile.TileContext(nc) as tc:
        with (
            tc.tile_pool(name="xpool", bufs=1) as xpool,
            tc.tile_pool(name="wpool", bufs=3) as wpool,
            tc.tile_pool(name="cpool", bufs=1) as cpool,
            tc.tile_pool(name="opool", bufs=4) as opool,
            tc.tile_pool(name="ppool", bufs=8, space="PSUM") as ppool,
        ):
            xt = xpool.tile([P, KO2, 2, TOK_SHARD], f8)
            dxt = xpool.tile([P, KO2, 2, TOK_SHARD], f8)
            bias_t = cpool.tile([P, O_TILES], f32)

            def load_wt(ot):
                wt = wpool.tile([P, KO2, 2, P], f8, tag="w")
                nc.sync.dma_start(out=wt[:], in_=w8d[ot, :, :, :, :])
                return wt

            # Prologue: first W chunk, then the resident x8/dx8 streams
            # (HWDGE issues in FIFO order), bias, then the remaining W
            # chunks stream behind.
            first_wt = load_wt(0)
            nc.sync.dma_start(out=xt[:], in_=x8d[:, :, :, :])
            nc.sync.dma_start(out=dxt[:], in_=dx8d[:, :, :, :])
            nc.sync.dma_start(out=bias_t[:], in_=biasd[:, :])

            for ot in range(O_TILES):
                wt = first_wt if ot == 0 else load_wt(ot)
                for tt in range(T_CHUNKS):
                    ps = ppool.tile([P, T_CHUNK], f32, tag="ps", name="ps")
                    for k in range(KO2):
                        nc.tensor.matmul(
                            ps[:],
                            wt[:, k, :, :],
                            xt[:, k, :, ts(tt, T_CHUNK)],
                            start=(k == 0),
                            stop=False,
                            perf_mode=DR,
                        )
                    for k in range(KO2):
                        nc.tensor.matmul(
                            ps[:],
                            wt[:, k, :, :],
                            dxt[:, k, :, ts(tt, T_CHUNK)],
                            start=False,
                            stop=(k == KO2 - 1),
                            perf_mode=DR,
                        )
                    ot_sb = opool.tile([P, T_CHUNK], f32, name="ot")
                    nc.scalar.activation(
                        ot_sb[:],
                        ps[:],
                        IDENT,
                        bias=bias_t[:, ds(ot, 1)],
                        scale=EVICT_SCALE,
                    )
                    nc.sync.dma_start(
                        out=out[ts(ot, P), ts(tt, T_CHUNK)], in_=ot_sb[:]
                    )

    nc.compile()
    return nc


def _make_in_maps(x, W, b, lora_A, lora_B):
    # LoRA merge: W' = W + scaling * B @ A  (exact fp32 host math)
    w_merged = W + SCALING * (lora_B @ lora_A)

    # W8[ki, o] = q(W'[o, ki] * SW); chunk layout [ot, p, k2, two, oi]
    wq = (w_merged.T * SW).astype(E4M3)
    w8 = np.ascontiguousarray(
        wq.reshape(KO2, 2, P, O_TILES, P).transpose(3, 2, 0, 1, 4)
    )

    xT = x.reshape(TOK, D_IN).T  # [D_IN, TOK]
    xq = (xT * SX).astype(E4M3)
    dxq = (xT * SX - xq.astype(np.float32)).astype(E4M3)

    bias = np.ascontiguousarray(b.reshape(O_TILES, P).T)  # [P, O_TILES]

    def shard(q8, i):
        s = q8[:, i * TOK_SHARD : (i + 1) * TOK_SHARD]
        return np.ascontiguousarray(
            s.reshape(KO2, 2, P, TOK_SHARD).transpose(2, 0, 1, 3)
        )

    return [
        {
            "x8": shard(xq, i),
            "dx8": shard(dxq, i),
            "w8": w8,
            "bias": bias,
        }
        for i in range(N_CORES)
    ]


def kernel(x, W, b, lora_A, lora_B):
    x = np.asarray(x, dtype=np.float32)
    W = np.asarray(W, dtype=np.float32)
    b = np.asarray(b, dtype=np.float32)
    lora_A = np.asarray(lora_A, dtype=np.float32)
    lora_B = np.asarray(lora_B, dtype=np.float32)

    if "main" not in _nc_cache:
        _nc_cache["main"] = _build()
    nc = _nc_cache["main"]

    in_maps = _make_in_maps(x, W, b, lora_A, lora_B)
    res = run_bass_kernel_spmd(nc, in_maps, list(range(N_CORES)))
    out = np.concatenate(
        [res.results[i]["outT"].T for i in range(N_CORES)], axis=0
    )
    return np.ascontiguousarray(out).reshape(B, S, D_OUT)


# revision 3
# speedup vs baseline: 1.4334x; 1.3527x over previous
"""LinearWithLoRA on 8 TRN2 NeuronCores — split-K fp8-DoubleRow/fp16 hybrid.

y = x @ W.T + b + 2.0 * (x @ A.T) @ B.T
  x: [4, 2048, 2048] f32, W: [2048, 2048], b: [2048], A: [16, 2048], B: [2048, 16]

Strategy:
- LoRA merge on host: W' = W + 2.0 * B @ A, so the device computes
  x @ W'.T + b. Data-parallel over tokens (8192 -> 1024/core), W'
  replicated, no collectives.
- The PE's fp8 DoubleRow mode (both operands e4m3) retires 2 k-planes per
  512-cycle instruction = 2x the bf16/fp16/fp32r rate. Pure fp8 is too
  noisy for the 2e-2 gate (2.41e-2), so split the contraction: 10 of the
  16 k-planes run as 5 DoubleRow fp8 instructions, the remaining 6 planes
  run exact in fp16. Same-scale quantization (x*32, W'*1024, powers of
  two so dequant is exact) lets both parts accumulate into one PSUM bank.
  Measured rel err 1.906e-2; PE cost = (5 + 6)/16 = 0.69 of a full-K
  one-dtype pass (~76us/core vs 109us floor for any single-dtype scheme).
- Operand swap: W chunks are stationary, x moving, so PSUM tiles are
  [out_ch(partition), tokens(free)] and the bias varies along partitions:
  eviction is ONE scalar-engine op per tile,
  out = Identity(psum * 2^-15 + bias[p]). Output is written transposed
  and re-transposed on host.
- All of W' (fp8+fp16 halves, ~5.5 MiB) and the x shard stay resident in
  SBUF. Matmuls are issued k-plane-outer over groups of 8 PSUM tiles so
  the PE consumes operands in exact DMA arrival order: w8 chunks for the
  first group, then x8 plane-by-plane, then w16/x16 — the PE starts after
  ~0.75 MiB instead of waiting for the full 8.25 MiB input stream.
"""

import numpy as np
import ml_dtypes

import concourse.bass as bass
import concourse.mybir as mybir
import concourse.tile as tile
from concourse import bacc
from concourse.bass import ds, ts
from concourse.bass_utils import run_bass_kernel_spmd

B, S, D_IN, D_OUT, R = 4, 2048, 2048, 2048, 16
SCALING = 32.0 / 16.0
N_CORES = 8
TOK = B * S  # 8192
TOK_SHARD = TOK // N_CORES  # 1024
P = 128
KF8 = 10  # k-planes (128 each) done in fp8 DoubleRow (must be even)
KF16 = 16 - KF8  # k-planes done in fp16
J8 = KF8 // 2  # DoubleRow instructions per tile
T_CHUNK = 512  # psum bank: 512 f32 per partition
T_CHUNKS = TOK_SHARD // T_CHUNK  # 2
O_TILES = D_OUT // P  # 16 out-channel tiles
OT_GROUP = 4  # out-tiles per psum group (x T_CHUNKS = 8 banks)

SX = 32.0  # x scale (2^5)
SW = 1024.0  # W scale (2^10)
EVICT_SCALE = 1.0 / (SX * SW)  # 2^-15, exact in fp32

E4M3 = ml_dtypes.float8_e4m3

_nc_cache = {}


def _build():
    f32 = mybir.dt.float32
    f8 = mybir.dt.float8e4
    f16 = mybir.dt.float16
    DR = mybir.MatmulPerfMode.DoubleRow
    IDENT = mybir.ActivationFunctionType.Identity

    nc = bacc.Bacc(None, target_bir_lowering=False)
    x8d = nc.dram_tensor("x8", [P, J8, 2, TOK_SHARD], f8, kind="ExternalInput")
    x16d = nc.dram_tensor("x16", [P, KF16, TOK_SHARD], f16, kind="ExternalInput")
    w8d = nc.dram_tensor("w8", [O_TILES, P, J8, 2, P], f8, kind="ExternalInput")
    w16d = nc.dram_tensor("w16", [O_TILES, P, KF16, P], f16, kind="ExternalInput")
    biasd = nc.dram_tensor("bias", [P, O_TILES], f32, kind="ExternalInput")
    out = nc.dram_tensor("outT", [D_OUT, TOK_SHARD], f32, kind="ExternalOutput")

    n_groups = O_TILES // OT_GROUP

    with tile.TileContext(nc) as tc:
        with (
            tc.tile_pool(name="xpool", bufs=1) as xpool,
            tc.tile_pool(name="wpool", bufs=1) as wpool,
            tc.tile_pool(name="cpool", bufs=1) as cpool,
            tc.tile_pool(name="opool", bufs=8) as opool,
            tc.tile_pool(name="ppool", bufs=8, space="PSUM") as ppool,
        ):
            x8t = xpool.tile([P, J8, 2, TOK_SHARD], f8)
            x16t = xpool.tile([P, KF16, TOK_SHARD], f16)
            w8t = wpool.tile([P, O_TILES, J8, 2, P], f8)
            w16t = wpool.tile([P, O_TILES, KF16, P], f16)
            bias_t = cpool.tile([P, O_TILES], f32)

            # DMA issue order == PE consumption order (HWDGE is FIFO):
            # group-0 fp8 weights, x8 plane-by-plane, group-0 fp16 weights,
            # x16 plane-by-plane, bias, then the remaining W chunks.
            for ot in range(OT_GROUP):
                nc.sync.dma_start(out=w8t[:, ot, :, :, :], in_=w8d[ot, :, :, :, :])
            for j in range(J8):
                nc.sync.dma_start(out=x8t[:, j, :, :], in_=x8d[:, j, :, :])
            for ot in range(OT_GROUP):
                nc.sync.dma_start(out=w16t[:, ot, :, :], in_=w16d[ot, :, :, :])
            for k in range(KF16):
                nc.sync.dma_start(out=x16t[:, k, :], in_=x16d[:, k, :])
            nc.sync.dma_start(out=bias_t[:], in_=biasd[:, :])
            for g in range(1, n_groups):
                for ot in range(g * OT_GROUP, (g + 1) * OT_GROUP):
                    nc.sync.dma_start(
                        out=w8t[:, ot, :, :, :], in_=w8d[ot, :, :, :, :]
                    )
                    nc.sync.dma_start(out=w16t[:, ot, :, :], in_=w16d[ot, :, :, :])

            for g in range(n_groups):
                tiles = [
                    (ot, tt)
                    for ot in range(g * OT_GROUP, (g + 1) * OT_GROUP)
                    for tt in range(T_CHUNKS)
                ]
                pss = [
                    ppool.tile([P, T_CHUNK], f32, tag="ps", name=f"ps{i}")
                    for i in range(len(tiles))
                ]
                # k-plane-outer: all 8 tiles advance together through the
                # operand stream, so a freshly arrived x plane feeds 8
                # back-to-back matmuls.
                for j in range(J8):
                    for i, (ot, tt) in enumerate(tiles):
                        nc.tensor.matmul(
                            pss[i][:],
                            w8t[:, ot, j, :, :],
                            x8t[:, j, :, ts(tt, T_CHUNK)],
                            start=(j == 0),
                            stop=False,
                            perf_mode=DR,
                        )
                for k in range(KF16):
                    for i, (ot, tt) in enumerate(tiles):
                        nc.tensor.matmul(
                            pss[i][:],
                            w16t[:, ot, k, :],
                            x16t[:, k, ts(tt, T_CHUNK)],
                            start=False,
                            stop=(k == KF16 - 1),
                        )
                for i, (ot, tt) in enumerate(tiles):
                    ot_sb = opool.tile([P, T_CHUNK], f32, name="ot")
                    nc.scalar.activation(
                        ot_sb[:],
                        pss[i][:],
                        IDENT,
                        bias=bias_t[:, ds(ot, 1)],
                        scale=EVICT_SCALE,
                    )
                    nc.sync.dma_start(
                        out=out[ts(ot, P), ts(tt, T_CHUNK)], in_=ot_sb[:]
                    )

    nc.compile()
    return nc


def _make_in_maps(x, W, b, lora_A, lora_B):
    # LoRA merge: W' = W + scaling * B @ A  (exact fp32 host math)
    w_merged = W + SCALING * (lora_B @ lora_A)

    KC = KF8 * P  # k cut point
    ws = w_merged.T * SW  # [D_IN, D_OUT]
    w8 = np.ascontiguousarray(
        ws[:KC].astype(E4M3).reshape(J8, 2, P, O_TILES, P).transpose(3, 2, 0, 1, 4)
    )
    w16 = np.ascontiguousarray(
        ws[KC:].astype(np.float16).reshape(KF16, P, O_TILES, P).transpose(2, 1, 0, 3)
    )

    xs = x.reshape(TOK, D_IN).T * SX  # [D_IN, TOK]
    xq8 = xs[:KC].astype(E4M3)
    xq16 = xs[KC:].astype(np.float16)

    bias = np.ascontiguousarray(b.reshape(O_TILES, P).T)  # [P, O_TILES]

    def shard8(i):
        s = xq8[:, i * TOK_SHARD : (i + 1) * TOK_SHARD]
        return np.ascontiguousarray(
            s.reshape(J8, 2, P, TOK_SHARD).transpose(2, 0, 1, 3)
        )

    def shard16(i):
        s = xq16[:, i * TOK_SHARD : (i + 1) * TOK_SHARD]
        return np.ascontiguousarray(
            s.reshape(KF16, P, TOK_SHARD).transpose(1, 0, 2)
        )

    return [
        {
            "x8": shard8(i),
            "x16": shard16(i),
            "w8": w8,
            "w16": w16,
            "bias": bias,
        }
        for i in range(N_CORES)
    ]


def kernel(x, W, b, lora_A, lora_B):
    x = np.asarray(x, dtype=np.float32)
    W = np.asarray(W, dtype=np.float32)
    b = np.asarray(b, dtype=np.float32)
    lora_A = np.asarray(lora_A, dtype=np.float32)
    lora_B = np.asarray(lora_B, dtype=np.float32)

    if "main" not in _nc_cache:
        _nc_cache["main"] = _build()
    nc = _nc_cache["main"]

    in_maps = _make_in_maps(x, W, b, lora_A, lora_B)
    res = run_bass_kernel_spmd(nc, in_maps, list(range(N_CORES)))
    out = np.concatenate(
        [res.results[i]["outT"].T for i in range(N_CORES)], axis=0
    )
    return np.ascontiguousarray(out).reshape(B, S, D_OUT)


# revision 5
# speedup vs baseline: 1.4421x; 1.0061x over previous
"""LinearWithLoRA on 8 TRN2 NeuronCores — split-K fp8-DoubleRow/fp16 hybrid.

y = x @ W.T + b + 2.0 * (x @ A.T) @ B.T
  x: [4, 2048, 2048] f32, W: [2048, 2048], b: [2048], A: [16, 2048], B: [2048, 16]

Strategy:
- LoRA merge on host: W' = W + 2.0 * B @ A, so the device computes
  x @ W'.T + b. Data-parallel over tokens (8192 -> 1024/core), W'
  replicated, no collectives.
- The PE's fp8 DoubleRow mode (both operands e4m3) retires 2 k-planes per
  512-cycle instruction = 2x the bf16/fp16/fp32r rate. Pure fp8 is too
  noisy for the 2e-2 gate (2.41e-2), so split the contraction: 10 of the
  16 k-planes run as 5 DoubleRow fp8 instructions, the remaining 6 planes
  run exact in fp16. Same-scale quantization (x*32, W'*1024, powers of
  two so dequant is exact) lets both parts accumulate into one PSUM bank.
  Measured rel err 1.906e-2; PE cost = (5 + 6)/16 = 0.69 of a full-K
  one-dtype pass (~76us/core vs 109us floor for any single-dtype scheme).
- Operand swap: W chunks are stationary, x moving, so PSUM tiles are
  [out_ch(partition), tokens(free)] and the bias varies along partitions:
  eviction is ONE scalar-engine op per tile,
  out = Identity(psum * 2^-15 + bias[p]). Output is written transposed
  and re-transposed on host.
- All of W' (fp8+fp16 halves, ~5.5 MiB) and the x shard stay resident in
  SBUF. Matmuls are issued k-plane-outer over groups of 8 PSUM tiles so
  the PE consumes operands in exact DMA arrival order: w8 chunks for the
  first group, then x8 plane-by-plane, then w16/x16 — the PE starts after
  ~0.75 MiB instead of waiting for the full 8.25 MiB input stream.
"""

import numpy as np
import ml_dtypes

import concourse.bass as bass
import concourse.mybir as mybir
import concourse.tile as tile
from concourse import bacc
from concourse.bass import ds, ts
from concourse.bass_utils import run_bass_kernel_spmd

B, S, D_IN, D_OUT, R = 4, 2048, 2048, 2048, 16
SCALING = 32.0 / 16.0
N_CORES = 8
TOK = B * S  # 8192
TOK_SHARD = TOK // N_CORES  # 1024
P = 128
KF8 = 10  # k-planes (128 each) done in fp8 DoubleRow (must be even)
KF16 = 16 - KF8  # k-planes done in fp16
J8 = KF8 // 2  # DoubleRow instructions per tile
T_CHUNK = 512  # psum bank: 512 f32 per partition
T_CHUNKS = TOK_SHARD // T_CHUNK  # 2
O_TILES = D_OUT // P  # 16 out-channel tiles
OT_GROUP = 4  # out-tiles per psum group (x T_CHUNKS = 8 banks)

SX = 32.0  # x scale (2^5)
SW = 1024.0  # W scale (2^10)
EVICT_SCALE = 1.0 / (SX * SW)  # 2^-15, exact in fp32

E4M3 = ml_dtypes.float8_e4m3

_nc_cache = {}


def _build():
    f32 = mybir.dt.float32
    f8 = mybir.dt.float8e4
    f16 = mybir.dt.float16
    DR = mybir.MatmulPerfMode.DoubleRow
    IDENT = mybir.ActivationFunctionType.Identity

    nc = bacc.Bacc(None, target_bir_lowering=False)
    x8d = nc.dram_tensor("x8", [P, J8, 2, TOK_SHARD], f8, kind="ExternalInput")
    x16d = nc.dram_tensor("x16", [P, KF16, TOK_SHARD], f16, kind="ExternalInput")
    w8d = nc.dram_tensor("w8", [O_TILES, P, J8, 2, P], f8, kind="ExternalInput")
    w16d = nc.dram_tensor("w16", [O_TILES, P, KF16, P], f16, kind="ExternalInput")
    biasd = nc.dram_tensor("bias", [P, O_TILES], f32, kind="ExternalInput")
    out = nc.dram_tensor("outT", [D_OUT, TOK_SHARD], f32, kind="ExternalOutput")

    n_groups = O_TILES // OT_GROUP

    with tile.TileContext(nc) as tc:
        with (
            tc.tile_pool(name="xpool", bufs=1) as xpool,
            tc.tile_pool(name="wpool", bufs=1) as wpool,
            tc.tile_pool(name="cpool", bufs=1) as cpool,
            tc.tile_pool(name="opool", bufs=8) as opool,
            tc.tile_pool(name="ppool", bufs=8, space="PSUM") as ppool,
        ):
            x8t = xpool.tile([P, J8, 2, TOK_SHARD], f8)
            x16t = xpool.tile([P, KF16, TOK_SHARD], f16)
            w8t = wpool.tile([P, O_TILES, J8, 2, P], f8)
            w16t = wpool.tile([P, O_TILES, KF16, P], f16)
            bias_t = cpool.tile([P, O_TILES], f32)

            # Partition-major views of the W inputs so a whole 4-chunk group
            # loads in ONE descriptor issue (the Sync engine issues DMA
            # descriptors serially at ~650ns each — issue count matters).
            w8r = w8d.rearrange("o p j t i -> p o j t i")
            w16r = w16d.rearrange("o p k i -> p o k i")

            def load_w_group(g):
                sl = ds(g * OT_GROUP, OT_GROUP)
                nc.sync.dma_start(out=w8t[:, sl, :, :, :], in_=w8r[:, sl, :, :, :])
                nc.sync.dma_start(out=w16t[:, sl, :, :], in_=w16r[:, sl, :, :])

            # DMA issue order == PE consumption order (HWDGE is FIFO):
            # group-0 fp8 weights, x8 plane-by-plane, group-0 fp16 weights,
            # x16 plane-by-plane, bias, then the remaining W groups.
            nc.sync.dma_start(
                out=w8t[:, ds(0, OT_GROUP), :, :, :],
                in_=w8r[:, ds(0, OT_GROUP), :, :, :],
            )
            for j in range(J8):
                nc.sync.dma_start(out=x8t[:, j, :, :], in_=x8d[:, j, :, :])
            nc.sync.dma_start(
                out=w16t[:, ds(0, OT_GROUP), :, :], in_=w16r[:, ds(0, OT_GROUP), :, :]
            )
            for k in range(KF16):
                nc.sync.dma_start(out=x16t[:, k, :], in_=x16d[:, k, :])
            nc.sync.dma_start(out=bias_t[:], in_=biasd[:, :])
            for g in range(1, n_groups):
                load_w_group(g)

            for g in range(n_groups):
                tiles = [
                    (ot, tt)
                    for ot in range(g * OT_GROUP, (g + 1) * OT_GROUP)
                    for tt in range(T_CHUNKS)
                ]
                pss = [
                    ppool.tile([P, T_CHUNK], f32, tag="ps", name=f"ps{i}")
                    for i in range(len(tiles))
                ]
                # k-plane-outer: all 8 tiles advance together through the
                # operand stream, so a freshly arrived x plane feeds 8
                # back-to-back matmuls.
                for j in range(J8):
                    for i, (ot, tt) in enumerate(tiles):
                        nc.tensor.matmul(
                            pss[i][:],
                            w8t[:, ot, j, :, :],
                            x8t[:, j, :, ts(tt, T_CHUNK)],
                            start=(j == 0),
                            stop=False,
                            perf_mode=DR,
                        )
                for k in range(KF16):
                    for i, (ot, tt) in enumerate(tiles):
                        nc.tensor.matmul(
                            pss[i][:],
                            w16t[:, ot, k, :],
                            x16t[:, k, ts(tt, T_CHUNK)],
                            start=False,
                            stop=(k == KF16 - 1),
                        )
                # Both T_CHUNK halves of an out-tile evict into one staging
                # tile and leave in a single 4KB/partition store.
                stage = {}
                for i, (ot, tt) in enumerate(tiles):
                    if ot not in stage:
                        stage[ot] = opool.tile([P, TOK_SHARD], f32, name="ot")
                    nc.scalar.activation(
                        stage[ot][:, ts(tt, T_CHUNK)],
                        pss[i][:],
                        IDENT,
                        bias=bias_t[:, ds(ot, 1)],
                        scale=EVICT_SCALE,
                    )
                for ot, st in stage.items():
                    nc.sync.dma_start(out=out[ts(ot, P), :], in_=st[:])

    nc.compile()
    return nc


def _make_in_maps(x, W, b, lora_A, lora_B):
    # LoRA merge: W' = W + scaling * B @ A  (exact fp32 host math)
    w_merged = W + SCALING * (lora_B @ lora_A)

    KC = KF8 * P  # k cut point
    ws = w_merged.T * SW  # [D_IN, D_OUT]
    w8 = np.ascontiguousarray(
        ws[:KC].astype(E4M3).reshape(J8, 2, P, O_TILES, P).transpose(3, 2, 0, 1, 4)
    )
    w16 = np.ascontiguousarray(
        ws[KC:].astype(np.float16).reshape(KF16, P, O_TILES, P).transpose(2, 1, 0, 3)
    )

    xs = x.reshape(TOK, D_IN).T * SX  # [D_IN, TOK]
    xq8 = xs[:KC].astype(E4M3)
    xq16 = xs[KC:].astype(np.float16)

    bias = np.ascontiguousarray(b.reshape(O_TILES, P).T)  # [P, O_TILES]

    def shard8(i):
        s = xq8[:, i * TOK_SHARD : (i + 1) * TOK_SHARD]
        return np.ascontiguousarray(
            s.reshape(J8, 2, P, TOK_SHARD).transpose(2, 0, 1, 3)
        )

    def shard16(i):
        s = xq16[:, i * TOK_SHARD : (i + 1) * TOK_SHARD]
        return np.ascontiguousarray(
            s.reshape(KF16, P, TOK_SHARD).transpose(1, 0, 2)
        )

    return [
        {
            "x8": shard8(i),
            "x16": shard16(i),
            "w8": w8,
            "w16": w16,
            "bias": bias,
        }
        for i in range(N_CORES)
    ]


def kernel(x, W, b, lora_A, lora_B):
    x = np.asarray(x, dtype=np.float32)
    W = np.asarray(W, dtype=np.float32)
    b = np.asarray(b, dtype=np.float32)
    lora_A = np.asarray(lora_A, dtype=np.float32)
    lora_B = np.asarray(lora_B, dtype=np.float32)

    if "main" not in _nc_cache:
        _nc_cache["main"] = _build()
    nc = _nc_cache["main"]

    in_maps = _make_in_maps(x, W, b, lora_A, lora_B)
    res = run_bass_kernel_spmd(nc, in_maps, list(range(N_CORES)))
    out = np.concatenate(
        [res.results[i]["outT"].T for i in range(N_CORES)], axis=0
    )
    return np.ascontiguousarray(out).reshape(B, S, D_OUT)


# revision 7
# speedup vs baseline: 1.5021x; 1.0416x over previous
"""LinearWithLoRA on 8 TRN2 NeuronCores — split-K fp8-DoubleRow/fp16 hybrid.

y = x @ W.T + b + 2.0 * (x @ A.T) @ B.T
  x: [4, 2048, 2048] f32, W: [2048, 2048], b: [2048], A: [16, 2048], B: [2048, 16]

Strategy:
- LoRA merge on host: W' = W + 2.0 * B @ A, so the device computes
  x @ W'.T + b. Data-parallel over tokens (8192 -> 1024/core), W'
  replicated, no collectives.
- The PE's fp8 DoubleRow mode (both operands e4m3) retires 2 k-planes per
  512-cycle instruction = 2x the bf16/fp16/fp32r rate. Pure fp8 is too
  noisy for the 2e-2 gate (2.41e-2), so split the contraction: 10 of the
  16 k-planes run as 5 DoubleRow fp8 instructions, the remaining 6 planes
  run exact in fp16. Same-scale quantization (x*32, W'*1024, powers of
  two so dequant is exact) lets both parts accumulate into one PSUM bank.
  Measured rel err 1.906e-2; PE cost = (5 + 6)/16 = 0.69 of a full-K
  one-dtype pass (~76us/core vs 109us floor for any single-dtype scheme).
- Operand swap: W chunks are stationary, x moving, so PSUM tiles are
  [out_ch(partition), tokens(free)] and the bias varies along partitions:
  eviction is ONE scalar-engine op per tile,
  out = Identity(psum * 2^-15 + bias[p]). Output is written transposed
  and re-transposed on host.
- All of W' (fp8+fp16 halves, ~5.5 MiB) and the x shard stay resident in
  SBUF. Matmuls are issued k-plane-outer over groups of 8 PSUM tiles so
  the PE consumes operands in exact DMA arrival order: w8 chunks for the
  first group, then x8 plane-by-plane, then w16/x16 — the PE starts after
  ~0.75 MiB instead of waiting for the full 8.25 MiB input stream.
"""

import numpy as np
import ml_dtypes

import concourse.bass as bass
import concourse.mybir as mybir
import concourse.tile as tile
from concourse import bacc
from concourse.bass import ds, ts
from concourse.bass_utils import run_bass_kernel_spmd

B, S, D_IN, D_OUT, R = 4, 2048, 2048, 2048, 16
SCALING = 32.0 / 16.0
N_CORES = 8
TOK = B * S  # 8192
TOK_SHARD = TOK // N_CORES  # 1024
P = 128
KF8 = 10  # k-planes (128 each) done in fp8 DoubleRow (must be even)
KF16 = 16 - KF8  # k-planes done in fp16
J8 = KF8 // 2  # DoubleRow instructions per tile
T_CHUNK = 512  # psum bank: 512 f32 per partition
T_CHUNKS = TOK_SHARD // T_CHUNK  # 2
O_TILES = D_OUT // P  # 16 out-channel tiles
OT_GROUP = 4  # out-tiles per psum group (x T_CHUNKS = 8 banks)

SX = 32.0  # x scale (2^5)
SW = 1024.0  # W scale (2^10)
EVICT_SCALE = 1.0 / (SX * SW)  # 2^-15, exact in fp32

E4M3 = ml_dtypes.float8_e4m3

_nc_cache = {}


def _build():
    f32 = mybir.dt.float32
    f8 = mybir.dt.float8e4
    f16 = mybir.dt.float16
    DR = mybir.MatmulPerfMode.DoubleRow
    IDENT = mybir.ActivationFunctionType.Identity

    nc = bacc.Bacc(None, target_bir_lowering=False)
    x8d = nc.dram_tensor("x8", [P, J8, 2, TOK_SHARD], f8, kind="ExternalInput")
    x16d = nc.dram_tensor("x16", [P, KF16, TOK_SHARD], f16, kind="ExternalInput")
    w8d = nc.dram_tensor("w8", [O_TILES, P, J8, 2, P], f8, kind="ExternalInput")
    w16d = nc.dram_tensor("w16", [O_TILES, P, KF16, P], f16, kind="ExternalInput")
    biasd = nc.dram_tensor("bias", [P, O_TILES], f32, kind="ExternalInput")
    out = nc.dram_tensor("outT", [D_OUT, TOK_SHARD], f32, kind="ExternalOutput")

    n_groups = O_TILES // OT_GROUP

    with tile.TileContext(nc) as tc:
        with (
            tc.tile_pool(name="xpool", bufs=1) as xpool,
            tc.tile_pool(name="wpool", bufs=1) as wpool,
            tc.tile_pool(name="cpool", bufs=1) as cpool,
            tc.tile_pool(name="opool", bufs=8) as opool,
            tc.tile_pool(name="ppool", bufs=8, space="PSUM") as ppool,
        ):
            x8t = xpool.tile([P, J8, 2, TOK_SHARD], f8)
            x16t = xpool.tile([P, KF16, TOK_SHARD], f16)
            w8t = wpool.tile([P, O_TILES, J8, 2, P], f8)
            w16t = wpool.tile([P, O_TILES, KF16, P], f16)
            bias_t = cpool.tile([P, O_TILES], f32)

            # Partition-major views of the W inputs so a whole 4-chunk group
            # loads in ONE descriptor issue (the Sync engine issues DMA
            # descriptors serially at ~650ns each — issue count matters).
            w8r = w8d.rearrange("o p j t i -> p o j t i")
            w16r = w16d.rearrange("o p k i -> p o k i")

            def load_w_group(g):
                sl = ds(g * OT_GROUP, OT_GROUP)
                nc.sync.dma_start(out=w8t[:, sl, :, :, :], in_=w8r[:, sl, :, :, :])
                nc.sync.dma_start(out=w16t[:, sl, :, :], in_=w16r[:, sl, :, :])

            # DMA issue order == PE consumption order (HWDGE is FIFO):
            # group-0 fp8 weights, x8 plane-by-plane, group-0 fp16 weights,
            # x16 plane-by-plane, bias, then the remaining W groups.
            nc.sync.dma_start(out=w8t[:, ds(0, 1), :, :, :], in_=w8r[:, ds(0, 1), :, :, :])
            nc.sync.dma_start(out=x8t[:, 0, :, :], in_=x8d[:, 0, :, :])
            nc.sync.dma_start(
                out=w8t[:, ds(1, OT_GROUP - 1), :, :, :],
                in_=w8r[:, ds(1, OT_GROUP - 1), :, :, :],
            )
            for j in range(1, J8):
                nc.sync.dma_start(out=x8t[:, j, :, :], in_=x8d[:, j, :, :])
            nc.sync.dma_start(
                out=w16t[:, ds(0, OT_GROUP), :, :], in_=w16r[:, ds(0, OT_GROUP), :, :]
            )
            for k in range(KF16):
                nc.sync.dma_start(out=x16t[:, k, :], in_=x16d[:, k, :])
            nc.sync.dma_start(out=bias_t[:], in_=biasd[:, :])
            for g in range(1, n_groups):
                load_w_group(g)

            def mm8(ps, ot, j, tt, start):
                nc.tensor.matmul(
                    ps[:],
                    w8t[:, ot, j, :, :],
                    x8t[:, j, :, ts(tt, T_CHUNK)],
                    start=start,
                    stop=False,
                    perf_mode=DR,
                )

            def mm16(ps, ot, k, tt):
                nc.tensor.matmul(
                    ps[:],
                    w16t[:, ot, k, :],
                    x16t[:, k, ts(tt, T_CHUNK)],
                    start=False,
                    stop=(k == KF16 - 1),
                )

            def evict(ps, st, ot, tt):
                nc.scalar.activation(
                    st[:, ts(tt, T_CHUNK)],
                    ps[:],
                    IDENT,
                    bias=bias_t[:, ds(ot, 1)],
                    scale=EVICT_SCALE,
                )

            for g in range(n_groups):
                tiles = [
                    (ot, tt)
                    for ot in range(g * OT_GROUP, (g + 1) * OT_GROUP)
                    for tt in range(T_CHUNKS)
                ]
                if g == 0:
                    # k-plane-outer: all 8 tiles advance together through
                    # the operand stream, consuming each freshly arrived x
                    # plane with 8 back-to-back matmuls (DMA-paced phase).
                    pss = [
                        ppool.tile([P, T_CHUNK], f32, tag="ps", name=f"ps{i}")
                        for i in range(len(tiles))
                    ]
                    for j in range(J8):
                        for i, (ot, tt) in enumerate(tiles):
                            mm8(pss[i], ot, j, tt, start=(j == 0))
                    for k in range(KF16):
                        for i, (ot, tt) in enumerate(tiles):
                            mm16(pss[i], ot, k, tt)
                    stage = {}
                    for i, (ot, tt) in enumerate(tiles):
                        if ot not in stage:
                            stage[ot] = opool.tile([P, TOK_SHARD], f32, name="ot")
                        evict(pss[i], stage[ot], ot, tt)
                    for ot, st in stage.items():
                        nc.sync.dma_start(out=out[ts(ot, P), :], in_=st[:])
                else:
                    # tile-major: each psum tile completes as early as
                    # possible so evictions and output stores stagger into
                    # the matmul stream instead of bunching at the end.
                    for ot in range(g * OT_GROUP, (g + 1) * OT_GROUP):
                        st = opool.tile([P, TOK_SHARD], f32, name="ot")
                        for tt in range(T_CHUNKS):
                            ps = ppool.tile([P, T_CHUNK], f32, tag="ps", name="ps")
                            for j in range(J8):
                                mm8(ps, ot, j, tt, start=(j == 0))
                            for k in range(KF16):
                                mm16(ps, ot, k, tt)
                            evict(ps, st, ot, tt)
                        nc.sync.dma_start(out=out[ts(ot, P), :], in_=st[:])

    nc.compile()
    return nc


def _make_in_maps(x, W, b, lora_A, lora_B):
    # LoRA merge: W' = W + scaling * B @ A  (exact fp32 host math)
    w_merged = W + SCALING * (lora_B @ lora_A)

    KC = KF8 * P  # k cut point
    ws = w_merged.T * SW  # [D_IN, D_OUT]
    w8 = np.ascontiguousarray(
        ws[:KC].astype(E4M3).reshape(J8, 2, P, O_TILES, P).transpose(3, 2, 0, 1, 4)
    )
    w16 = np.ascontiguousarray(
        ws[KC:].astype(np.float16).reshape(KF16, P, O_TILES, P).transpose(2, 1, 0, 3)
    )

    xs = x.reshape(TOK, D_IN).T * SX  # [D_IN, TOK]
    xq8 = xs[:KC].astype(E4M3)
    xq16 = xs[KC:].astype(np.float16)

    bias = np.ascontiguousarray(b.reshape(O_TILES, P).T)  # [P, O_TILES]

    def shard8(i):
        s = xq8[:, i * TOK_SHARD : (i + 1) * TOK_SHARD]
        return np.ascontiguousarray(
            s.reshape(J8, 2, P, TOK_SHARD).transpose(2, 0, 1, 3)
        )

    def shard16(i):
        s = xq16[:, i * TOK_SHARD : (i + 1) * TOK_SHARD]
        return np.ascontiguousarray(
            s.reshape(KF16, P, TOK_SHARD).transpose(1, 0, 2)
        )

    return [
        {
            "x8": shard8(i),
            "x16": shard16(i),
            "w8": w8,
            "w16": w16,
            "bias": bias,
        }
        for i in range(N_CORES)
    ]


def kernel(x, W, b, lora_A, lora_B):
    x = np.asarray(x, dtype=np.float32)
    W = np.asarray(W, dtype=np.float32)
    b = np.asarray(b, dtype=np.float32)
    lora_A = np.asarray(lora_A, dtype=np.float32)
    lora_B = np.asarray(lora_B, dtype=np.float32)

    if "main" not in _nc_cache:
        _nc_cache["main"] = _build()
    nc = _nc_cache["main"]

    in_maps = _make_in_maps(x, W, b, lora_A, lora_B)
    res = run_bass_kernel_spmd(nc, in_maps, list(range(N_CORES)))
    out = np.concatenate(
        [res.results[i]["outT"].T for i in range(N_CORES)], axis=0
    )
    return np.ascontiguousarray(out).reshape(B, S, D_OUT)
